# revision 9
# baseline (speedup 1.0000x reference)
"""Trainium2 Bass kernel for nn_Episode_91164975825422.

Computation (see module docstring in reference):
  - e1 = table[heads], e2 = table[tails]    (embedding gathers, [B,256])
  - s1 = pearson(context, e1), s2 = pearson(context, e2)
  - start_entities = where(s1 > s2, heads, tails) as f32
  - softmax/argmax over logits[-1] -> last_scores = 1/sum(exp(l-max))
  - chosen = action_space[-1][b, argmax_b, 1:4] -> current_entities/timestamps

Sharding: pure data parallel, batch B=65536 split over 8 cores (8192 rows
each); the [V,D] embedding table is replicated. Only the last timestep of
logits / action_space is used by the reference, so only that slice is sent
to the devices.

Row mapping on each core: row = p*64 + c  (p = SBUF partition, c = column).

Engine split (per core), each ~130us so the pipeline stays balanced:
  GpSimd  : 128 indirect row gathers (serial SWDGE descriptor-gen floor)
  DVE     : bn_stats for x mean+var, mul+reduce for sum(x*e1)/sum(x*e2),
            softmax max/denoms/argmax-mask, masked chosen-field sums
  ACT     : e1/e2 sums and sums-of-squares (Copy/Square with accumulate),
            exp, int->float conversion of action_space
  Sync DMA: all direct loads/stores (HWDGE)
(tensor_tensor_reduce would fuse mul+reduce but faults this runtime.)
"""

import sys

sys.path.insert(0, "/opt/trn_rl_repo")

import numpy as np
from contextlib import ExitStack

import concourse.bass as bass
import concourse.tile as tile
from concourse import bacc, mybir

N_CORES = 8
B = 65536
D = 256
V = 125726
A = 50
T = 4
P = 128

B_LOC = B // N_CORES          # 8192 rows per core
C = B_LOC // P                # 64 row-columns per partition
SUB = 8                       # rows-per-partition processed per group
GROUPS = C // SUB             # 8 groups

F32 = mybir.dt.float32
I32 = mybir.dt.int32
AX = mybir.AxisListType
OP = mybir.AluOpType
AF = mybir.ActivationFunctionType


def _bcast(ap, axis, count):
    """Insert a stride-0 (broadcast) axis into an AP at position `axis`."""
    return bass.AP(
        tensor=ap.tensor,
        offset=ap.offset,
        ap=list(ap.ap[:axis]) + [[0, count]] + list(ap.ap[axis:]),
    )


def build_kernel(b_loc=B_LOC, sub=SUB):
    cols = b_loc // P
    groups = cols // sub
    nc = bacc.Bacc("TRN2", target_bir_lowering=False, debug=False,
                   num_devices=N_CORES)

    x_d = nc.dram_tensor("x", [b_loc, D], F32, kind="ExternalInput").ap()
    tab_d = nc.dram_tensor("table", [V, D], F32, kind="ExternalInput").ap()
    hd_d = nc.dram_tensor("heads", [b_loc], I32, kind="ExternalInput").ap()
    tl_d = nc.dram_tensor("tails", [b_loc], I32, kind="ExternalInput").ap()
    lg_d = nc.dram_tensor("lg", [b_loc, A], F32, kind="ExternalInput").ap()
    as_d = nc.dram_tensor("asl", [b_loc, 4 * A], I32, kind="ExternalInput").ap()
    outf = nc.dram_tensor("outf", [5, b_loc], F32, kind="ExternalOutput").ap()
    outi = nc.dram_tensor("outi", [2, b_loc], I32, kind="ExternalOutput").ap()

    x_r = x_d.rearrange("(p c) d -> p c d", p=P)
    lg_r = lg_d.rearrange("(p c) a -> p c a", p=P)
    as_r = as_d.rearrange("(p c) k -> p c k", p=P)
    hd_r = hd_d.rearrange("(p c) -> p c", p=P)
    tl_r = tl_d.rearrange("(p c) -> p c", p=P)
    outf_r = outf.rearrange("o (p c) -> o p c", p=P)
    outi_r = outi.rearrange("o (p c) -> o p c", p=P)

    with tile.TileContext(nc) as tc, ExitStack() as ctx:
        persist = ctx.enter_context(tc.tile_pool(name="persist", bufs=1))
        gin = ctx.enter_context(tc.tile_pool(name="gin", bufs=3))
        gtmp = ctx.enter_context(tc.tile_pool(name="gtmp", bufs=2))
        prodp = ctx.enter_context(tc.tile_pool(name="prod", bufs=4))
        fin = ctx.enter_context(tc.tile_pool(name="fin", bufs=1))

        heads_sb = persist.tile([P, cols], I32)
        tails_sb = persist.tile([P, cols], I32)
        nc.sync.dma_start(out=heads_sb[:], in_=hd_r)
        nc.sync.dma_start(out=tails_sb[:], in_=tl_r)

        xst = persist.tile([P, cols, 6], F32)     # bn_stats of context rows
        sy1 = persist.tile([P, cols], F32)        # ACT: sum(e1) per row
        syy1 = persist.tile([P, cols], F32)       # ACT: sum(e1^2) per row
        sy2 = persist.tile([P, cols], F32)        # ACT: sum(e2) per row
        syy2 = persist.tile([P, cols], F32)       # ACT: sum(e2^2) per row
        sxy1 = persist.tile([P, cols], F32)       # sum x*e1 per row
        sxy2 = persist.tile([P, cols], F32)
        zsum = persist.tile([P, cols], F32)       # softmax denominators
        chosen = persist.tile([P, cols, 3], F32)  # action fields 1..3 at argmax

        for g in range(groups):
            cs = slice(g * sub, (g + 1) * sub)

            xg = gin.tile([P, sub, D], F32, tag="xg")
            nc.sync.dma_start(out=xg[:], in_=x_r[:, cs, :])
            # HW only supports one offset per partition per indirect DMA
            e1g = gin.tile([P, sub, D], F32, tag="e1g")
            e2g = gin.tile([P, sub, D], F32, tag="e2g")
            for s in range(sub):
                c0 = g * sub + s
                nc.gpsimd.indirect_dma_start(
                    out=e1g[:, s, :], out_offset=None, in_=tab_d,
                    in_offset=bass.IndirectOffsetOnAxis(
                        ap=heads_sb[:, c0:c0 + 1], axis=0),
                )
                nc.gpsimd.indirect_dma_start(
                    out=e2g[:, s, :], out_offset=None, in_=tab_d,
                    in_offset=bass.IndirectOffsetOnAxis(
                        ap=tails_sb[:, c0:c0 + 1], axis=0),
                )
            lgt = gin.tile([P, sub, A], F32, tag="lgt")
            nc.sync.dma_start(out=lgt[:], in_=lg_r[:, cs, :])
            asgi = gin.tile([P, sub, 4 * A], I32, tag="asgi")
            nc.sync.dma_start(out=asgi[:], in_=as_r[:, cs, :])

            # --- Pearson stats ---
            # e1/e2 sums + squares on the Scalar engine, x stats on DVE
            for h in range(sub):
                c0 = g * sub + h
                nc.vector.bn_stats(out=xst[:, c0, :], in_=xg[:, h, :])
                sq1 = prodp.tile([P, D], F32, tag="sq")
                nc.scalar.activation(out=sq1[:], in_=e1g[:, h, :],
                                     func=AF.Square,
                                     accum_out=syy1[:, c0:c0 + 1])
                cp1 = prodp.tile([P, D], F32, tag="sq")
                nc.scalar.activation(out=cp1[:], in_=e1g[:, h, :],
                                     func=AF.Copy,
                                     accum_out=sy1[:, c0:c0 + 1])
                sq2 = prodp.tile([P, D], F32, tag="sq")
                nc.scalar.activation(out=sq2[:], in_=e2g[:, h, :],
                                     func=AF.Square,
                                     accum_out=syy2[:, c0:c0 + 1])
                cp2 = prodp.tile([P, D], F32, tag="sq")
                nc.scalar.activation(out=cp2[:], in_=e2g[:, h, :],
                                     func=AF.Copy,
                                     accum_out=sy2[:, c0:c0 + 1])
            # sum(x*e): elementwise product into scratch (keeps DVE
            # independent of the ACT stat passes) then 3D row-reduce
            xf = xg[:].rearrange("p s d -> p (s d)")
            pr1 = gtmp.tile([P, sub, D], F32, tag="pr")
            nc.vector.tensor_tensor(out=pr1[:].rearrange("p s d -> p (s d)"),
                                    in0=xf, in1=e1g[:].rearrange("p s d -> p (s d)"),
                                    op=OP.mult)
            nc.vector.tensor_reduce(out=sxy1[:, cs], in_=pr1[:], axis=AX.X,
                                    op=OP.add)
            pr2 = gtmp.tile([P, sub, D], F32, tag="pr")
            nc.vector.tensor_tensor(out=pr2[:].rearrange("p s d -> p (s d)"),
                                    in0=xf, in1=e2g[:].rearrange("p s d -> p (s d)"),
                                    op=OP.mult)
            nc.vector.tensor_reduce(out=sxy2[:, cs], in_=pr2[:], axis=AX.X,
                                    op=OP.add)

            # --- softmax max / denom / argmax mask ---
            mg = gtmp.tile([P, sub], F32, tag="mg")
            nc.vector.tensor_reduce(out=mg[:], in_=lgt[:], axis=AX.X, op=OP.max)
            mg_b = _bcast(mg[:], 2, A)
            lsub = gtmp.tile([P, sub, A], F32, tag="lsub")
            nc.vector.tensor_tensor(out=lsub[:], in0=lgt[:], in1=mg_b,
                                    op=OP.subtract)
            ex = gtmp.tile([P, sub, A], F32, tag="ex")
            nc.scalar.activation(out=ex[:], in_=lsub[:], func=AF.Exp)
            nc.vector.tensor_reduce(out=zsum[:, cs], in_=ex[:], axis=AX.X,
                                    op=OP.add)
            eq = gtmp.tile([P, sub, A], F32, tag="eq")
            nc.vector.tensor_scalar(out=eq[:], in0=lsub[:], scalar1=0.0,
                                    scalar2=None, op0=OP.is_equal)

            # --- chosen action fields via masked sum ---
            # asgi viewed as [P, sub, field k(4), action a(50)]; take k=1..3.
            # int32 in0 with f32 in1/out works natively on DVE.
            as_kv = asgi[:].rearrange("p s (a k) -> p s k a", k=4)[:, :, 1:4, :]
            eq_b = _bcast(eq[:], 2, 3)
            tt3 = gtmp.tile([P, sub, 3, A], F32, tag="tt3")
            nc.vector.tensor_tensor(out=tt3[:], in0=as_kv, in1=eq_b, op=OP.mult)
            nc.vector.tensor_reduce(out=chosen[:, cs, :], in_=tt3[:], axis=AX.X,
                                    op=OP.add)

        # ---------------- final per-row math on [P, cols] ----------------
        _fc = [0]

        def ftile():
            _fc[0] += 1
            return fin.tile([P, cols], F32, name=f"fin{_fc[0]}",
                            tag=f"fin{_fc[0]}")

        def mean_var(st):
            me, mo = st[:, :, 1], st[:, :, 4]
            m2e, m2o = st[:, :, 2], st[:, :, 5]
            mean = ftile()
            nc.vector.tensor_tensor(out=mean[:], in0=me, in1=mo, op=OP.add)
            nc.vector.tensor_scalar_mul(mean[:], mean[:], 0.5)
            dm = ftile()
            nc.vector.tensor_tensor(out=dm[:], in0=me, in1=mo, op=OP.subtract)
            nc.vector.tensor_tensor(out=dm[:], in0=dm[:], in1=dm[:], op=OP.mult)
            var = ftile()
            nc.vector.tensor_tensor(out=var[:], in0=m2e, in1=m2o, op=OP.add)
            nc.vector.tensor_scalar_mul(var[:], var[:], 1.0 / D)
            nc.vector.tensor_scalar_mul(dm[:], dm[:], 0.25)
            nc.vector.tensor_tensor(out=var[:], in0=var[:], in1=dm[:], op=OP.add)
            return mean, var

        mx, vx = mean_var(xst)

        def mean_var_sums(sy, syy):
            # mean/var from ACT-accumulated sums: m = S/D, v = SS/D - m^2
            m = ftile()
            nc.vector.tensor_scalar_mul(m[:], sy[:], 1.0 / D)
            v = ftile()
            nc.vector.tensor_scalar_mul(v[:], syy[:], 1.0 / D)
            msq = ftile()
            nc.vector.tensor_tensor(out=msq[:], in0=m[:], in1=m[:], op=OP.mult)
            nc.vector.tensor_tensor(out=v[:], in0=v[:], in1=msq[:],
                                    op=OP.subtract)
            return m, v

        m1, v1 = mean_var_sums(sy1, syy1)
        m2, v2 = mean_var_sums(sy2, syy2)

        def pearson(sxy, my, vy):
            num = ftile()
            nc.vector.tensor_scalar_mul(num[:], sxy[:], 1.0 / D)
            t = ftile()
            nc.vector.tensor_tensor(out=t[:], in0=mx[:], in1=my[:], op=OP.mult)
            nc.vector.tensor_tensor(out=num[:], in0=num[:], in1=t[:],
                                    op=OP.subtract)
            den = ftile()
            nc.vector.tensor_tensor(out=den[:], in0=vx[:], in1=vy[:], op=OP.mult)
            nc.scalar.sqrt(den[:], den[:])
            nc.vector.reciprocal(out=den[:], in_=den[:])
            nc.vector.tensor_tensor(out=num[:], in0=num[:], in1=den[:],
                                    op=OP.mult)
            return num

        s1v = pearson(sxy1, m1, v1)
        s2v = pearson(sxy2, m2, v2)

        cmp = ftile()
        nc.vector.tensor_tensor(out=cmp[:], in0=s1v[:], in1=s2v[:], op=OP.is_gt)
        headsf = ftile()
        nc.vector.tensor_copy(out=headsf[:], in_=heads_sb[:])
        tailsf = ftile()
        nc.vector.tensor_copy(out=tailsf[:], in_=tails_sb[:])
        # arithmetic blend: start = tails + (heads - tails) * (s1 > s2)
        # exact: cmp is 0/1 and entity ids are < 2^24
        startv = ftile()
        nc.vector.tensor_tensor(out=startv[:], in0=headsf[:], in1=tailsf[:],
                                op=OP.subtract)
        nc.vector.tensor_tensor(out=startv[:], in0=startv[:], in1=cmp[:],
                                op=OP.mult)
        nc.vector.tensor_tensor(out=startv[:], in0=startv[:], in1=tailsf[:],
                                op=OP.add)

        lsc = ftile()
        nc.vector.reciprocal(out=lsc[:], in_=zsum[:])

        ts1i = fin.tile([P, cols], I32)
        nc.vector.tensor_copy(out=ts1i[:], in_=chosen[:, :, 1])
        ts2i = fin.tile([P, cols], I32)
        nc.vector.tensor_copy(out=ts2i[:], in_=chosen[:, :, 2])

        nc.sync.dma_start(out=outf_r[0], in_=s1v[:])
        nc.sync.dma_start(out=outf_r[1], in_=s2v[:])
        nc.sync.dma_start(out=outf_r[2], in_=lsc[:])
        nc.sync.dma_start(out=outf_r[3], in_=startv[:])
        nc.sync.dma_start(out=outf_r[4], in_=chosen[:, :, 0])
        nc.sync.dma_start(out=outi_r[0], in_=ts1i[:])
        nc.sync.dma_start(out=outi_r[1], in_=ts2i[:])

    nc.compile()
    return nc


_NC = None


def _get_nc():
    global _NC
    if _NC is None:
        _NC = build_kernel()
    return _NC


def kernel(context_qa, embedding_table, heads, tails, action_space, logits):
    from concourse.bass_utils import run_bass_kernel_spmd

    ctx = np.ascontiguousarray(np.asarray(context_qa, dtype=np.float32))
    table = np.ascontiguousarray(np.asarray(embedding_table, dtype=np.float32))
    heads_i = np.ascontiguousarray(np.asarray(heads).astype(np.int32))
    tails_i = np.ascontiguousarray(np.asarray(tails).astype(np.int32))
    lg = np.ascontiguousarray(np.asarray(logits, dtype=np.float32)[T - 1])
    asl = np.ascontiguousarray(
        np.asarray(action_space, dtype=np.int32)[T - 1].reshape(B, 4 * A))

    nc = _get_nc()
    in_maps = []
    for c in range(N_CORES):
        sl = slice(c * B_LOC, (c + 1) * B_LOC)
        in_maps.append({
            "x": ctx[sl], "table": table, "heads": heads_i[sl],
            "tails": tails_i[sl], "lg": lg[sl], "asl": asl[sl],
        })
    res = run_bass_kernel_spmd(nc, in_maps, list(range(N_CORES)))

    outf = np.concatenate([res.results[c]["outf"] for c in range(N_CORES)],
                          axis=1)
    outi = np.concatenate([res.results[c]["outi"] for c in range(N_CORES)],
                          axis=1)
    out = outf[:3].astype(np.float32)
    start_entities = outf[3].astype(np.float32)
    current_entities = outf[4].astype(np.float32)
    current_timestamps = outi[0].astype(np.int32)
    current_timestamps2 = outi[1].astype(np.int32)
    return (out, start_entities, current_entities, current_timestamps,
            current_timestamps2)


# revision 10
# speedup vs baseline: 1.2188x; 1.2188x over previous
"""Trainium2 Bass kernel for nn_Episode_91164975825422.

Computation (see module docstring in reference):
  - e1 = table[heads], e2 = table[tails]    (embedding gathers, [B,256])
  - s1 = pearson(context, e1), s2 = pearson(context, e2)
  - start_entities = where(s1 > s2, heads, tails) as f32
  - softmax/argmax over logits[-1] -> last_scores = 1/sum(exp(l-max))
  - chosen = action_space[-1][b, argmax_b, 1:4] -> current_entities/timestamps

Sharding: pure data parallel, batch B=65536 split over 8 cores (8192 rows
each); the [V,D] embedding table is replicated. Only the last timestep of
logits / action_space is used by the reference, so only that slice is sent
to the devices.

Row mapping on each core: row = p*64 + c  (p = SBUF partition, c = column).

Engine split (per core), each ~130us so the pipeline stays balanced:
  GpSimd  : 128 indirect row gathers (serial SWDGE descriptor-gen floor)
  DVE     : bn_stats for x mean+var, mul+reduce for sum(x*e1)/sum(x*e2),
            softmax max/denoms/argmax-mask, masked chosen-field sums
  ACT     : e1/e2 sums and sums-of-squares (Copy/Square with accumulate),
            exp, int->float conversion of action_space
  Sync DMA: all direct loads/stores (HWDGE)
(tensor_tensor_reduce would fuse mul+reduce but faults this runtime.)
"""

import sys

sys.path.insert(0, "/opt/trn_rl_repo")

import numpy as np
from contextlib import ExitStack

import concourse.bass as bass
import concourse.tile as tile
from concourse import bacc, mybir

N_CORES = 8
B = 65536
D = 256
V = 125726
A = 50
T = 4
P = 128

B_LOC = B // N_CORES          # 8192 rows per core
C = B_LOC // P                # 64 row-columns per partition
SUB = 8                       # rows-per-partition processed per group
GROUPS = C // SUB             # 8 groups

F32 = mybir.dt.float32
I32 = mybir.dt.int32
AX = mybir.AxisListType
OP = mybir.AluOpType
AF = mybir.ActivationFunctionType


def _bcast(ap, axis, count):
    """Insert a stride-0 (broadcast) axis into an AP at position `axis`."""
    return bass.AP(
        tensor=ap.tensor,
        offset=ap.offset,
        ap=list(ap.ap[:axis]) + [[0, count]] + list(ap.ap[axis:]),
    )


def build_kernel(b_loc=B_LOC, sub=SUB):
    cols = b_loc // P
    groups = cols // sub
    nc = bacc.Bacc("TRN2", target_bir_lowering=False, debug=False,
                   num_devices=N_CORES)

    x_d = nc.dram_tensor("x", [b_loc, D], F32, kind="ExternalInput").ap()
    tab_d = nc.dram_tensor("table", [V, D], F32, kind="ExternalInput").ap()
    hd_d = nc.dram_tensor("heads", [b_loc], I32, kind="ExternalInput").ap()
    tl_d = nc.dram_tensor("tails", [b_loc], I32, kind="ExternalInput").ap()
    lg_d = nc.dram_tensor("lg", [b_loc, A], F32, kind="ExternalInput").ap()
    as_d = nc.dram_tensor("asl", [b_loc, 4 * A], I32, kind="ExternalInput").ap()
    # outputs packed [P, n, cols] so the store is one contiguous-per-
    # partition DMA; the host unpacks (row = p*cols + c)
    outf = nc.dram_tensor("outf", [P, 5, b_loc // P], F32,
                          kind="ExternalOutput").ap()
    outi = nc.dram_tensor("outi", [P, 2, b_loc // P], I32,
                          kind="ExternalOutput").ap()

    x_r = x_d.rearrange("(p c) d -> p c d", p=P)
    lg_r = lg_d.rearrange("(p c) a -> p c a", p=P)
    as_r = as_d.rearrange("(p c) k -> p c k", p=P)
    hd_r = hd_d.rearrange("(p c) -> p c", p=P)
    tl_r = tl_d.rearrange("(p c) -> p c", p=P)

    with tile.TileContext(nc) as tc, ExitStack() as ctx:
        persist = ctx.enter_context(tc.tile_pool(name="persist", bufs=1))
        gin = ctx.enter_context(tc.tile_pool(name="gin", bufs=3))
        gtmp = ctx.enter_context(tc.tile_pool(name="gtmp", bufs=2))
        prodp = ctx.enter_context(tc.tile_pool(name="prod", bufs=4))
        fin = ctx.enter_context(tc.tile_pool(name="fin", bufs=1))

        heads_sb = persist.tile([P, cols], I32)
        tails_sb = persist.tile([P, cols], I32)
        nc.sync.dma_start(out=heads_sb[:], in_=hd_r)
        nc.sync.dma_start(out=tails_sb[:], in_=tl_r)

        xst = persist.tile([P, cols, 6], F32)     # bn_stats of context rows
        sy1 = persist.tile([P, cols], F32)        # ACT: sum(e1) per row
        syy1 = persist.tile([P, cols], F32)       # ACT: sum(e1^2) per row
        sy2 = persist.tile([P, cols], F32)        # ACT: sum(e2) per row
        syy2 = persist.tile([P, cols], F32)       # ACT: sum(e2^2) per row
        sxy1 = persist.tile([P, cols], F32)       # sum x*e1 per row
        sxy2 = persist.tile([P, cols], F32)
        zsum = persist.tile([P, cols], F32)       # softmax denominators
        chosen = persist.tile([P, cols, 3], F32)  # action fields 1..3 at argmax

        for g in range(groups):
            cs = slice(g * sub, (g + 1) * sub)

            xg = gin.tile([P, sub, D], F32, tag="xg")
            nc.sync.dma_start(out=xg[:], in_=x_r[:, cs, :])
            # HW only supports one offset per partition per indirect DMA
            e1g = gin.tile([P, sub, D], F32, tag="e1g")
            e2g = gin.tile([P, sub, D], F32, tag="e2g")
            for s in range(sub):
                c0 = g * sub + s
                nc.gpsimd.indirect_dma_start(
                    out=e1g[:, s, :], out_offset=None, in_=tab_d,
                    in_offset=bass.IndirectOffsetOnAxis(
                        ap=heads_sb[:, c0:c0 + 1], axis=0),
                )
                nc.gpsimd.indirect_dma_start(
                    out=e2g[:, s, :], out_offset=None, in_=tab_d,
                    in_offset=bass.IndirectOffsetOnAxis(
                        ap=tails_sb[:, c0:c0 + 1], axis=0),
                )
            lgt = gin.tile([P, sub, A], F32, tag="lgt")
            nc.sync.dma_start(out=lgt[:], in_=lg_r[:, cs, :])
            asgi = gin.tile([P, sub, 4 * A], I32, tag="asgi")
            nc.sync.dma_start(out=asgi[:], in_=as_r[:, cs, :])

            # --- Pearson stats ---
            # e1/e2 sums + squares on the Scalar engine, x stats on DVE
            for h in range(sub):
                c0 = g * sub + h
                nc.vector.bn_stats(out=xst[:, c0, :], in_=xg[:, h, :])
                sq1 = prodp.tile([P, D], F32, tag="sq")
                nc.scalar.activation(out=sq1[:], in_=e1g[:, h, :],
                                     func=AF.Square,
                                     accum_out=syy1[:, c0:c0 + 1])
                cp1 = prodp.tile([P, D], F32, tag="sq")
                nc.scalar.activation(out=cp1[:], in_=e1g[:, h, :],
                                     func=AF.Copy,
                                     accum_out=sy1[:, c0:c0 + 1])
                sq2 = prodp.tile([P, D], F32, tag="sq")
                nc.scalar.activation(out=sq2[:], in_=e2g[:, h, :],
                                     func=AF.Square,
                                     accum_out=syy2[:, c0:c0 + 1])
                cp2 = prodp.tile([P, D], F32, tag="sq")
                nc.scalar.activation(out=cp2[:], in_=e2g[:, h, :],
                                     func=AF.Copy,
                                     accum_out=sy2[:, c0:c0 + 1])
            # sum(x*e): elementwise product into scratch (keeps DVE
            # independent of the ACT stat passes) then 3D row-reduce
            xf = xg[:].rearrange("p s d -> p (s d)")
            pr1 = gtmp.tile([P, sub, D], F32, tag="pr")
            nc.vector.tensor_tensor(out=pr1[:].rearrange("p s d -> p (s d)"),
                                    in0=xf, in1=e1g[:].rearrange("p s d -> p (s d)"),
                                    op=OP.mult)
            nc.vector.tensor_reduce(out=sxy1[:, cs], in_=pr1[:], axis=AX.X,
                                    op=OP.add)
            pr2 = gtmp.tile([P, sub, D], F32, tag="pr")
            nc.vector.tensor_tensor(out=pr2[:].rearrange("p s d -> p (s d)"),
                                    in0=xf, in1=e2g[:].rearrange("p s d -> p (s d)"),
                                    op=OP.mult)
            nc.vector.tensor_reduce(out=sxy2[:, cs], in_=pr2[:], axis=AX.X,
                                    op=OP.add)

            # --- softmax max / denom / argmax mask ---
            mg = gtmp.tile([P, sub], F32, tag="mg")
            nc.vector.tensor_reduce(out=mg[:], in_=lgt[:], axis=AX.X, op=OP.max)
            mg_b = _bcast(mg[:], 2, A)
            lsub = gtmp.tile([P, sub, A], F32, tag="lsub")
            nc.vector.tensor_tensor(out=lsub[:], in0=lgt[:], in1=mg_b,
                                    op=OP.subtract)
            ex = gtmp.tile([P, sub, A], F32, tag="ex")
            nc.scalar.activation(out=ex[:], in_=lsub[:], func=AF.Exp)
            nc.vector.tensor_reduce(out=zsum[:, cs], in_=ex[:], axis=AX.X,
                                    op=OP.add)
            eq = gtmp.tile([P, sub, A], F32, tag="eq")
            nc.vector.tensor_scalar(out=eq[:], in0=lsub[:], scalar1=0.0,
                                    scalar2=None, op0=OP.is_equal)

            # --- chosen action fields via masked sum ---
            # asgi viewed as [P, sub, field k(4), action a(50)]; take k=1..3.
            # int32 in0 with f32 in1/out works natively on DVE.
            as_kv = asgi[:].rearrange("p s (a k) -> p s k a", k=4)[:, :, 1:4, :]
            eq_b = _bcast(eq[:], 2, 3)
            tt3 = gtmp.tile([P, sub, 3, A], F32, tag="tt3")
            nc.vector.tensor_tensor(out=tt3[:], in0=as_kv, in1=eq_b, op=OP.mult)
            nc.vector.tensor_reduce(out=chosen[:, cs, :], in_=tt3[:], axis=AX.X,
                                    op=OP.add)

        # ---------------- final per-row math on [P, cols] ----------------
        _fc = [0]

        def ftile():
            _fc[0] += 1
            return fin.tile([P, cols], F32, name=f"fin{_fc[0]}",
                            tag=f"fin{_fc[0]}")

        def mean_var(st):
            me, mo = st[:, :, 1], st[:, :, 4]
            m2e, m2o = st[:, :, 2], st[:, :, 5]
            mean = ftile()
            nc.vector.tensor_tensor(out=mean[:], in0=me, in1=mo, op=OP.add)
            nc.vector.tensor_scalar_mul(mean[:], mean[:], 0.5)
            dm = ftile()
            nc.vector.tensor_tensor(out=dm[:], in0=me, in1=mo, op=OP.subtract)
            nc.vector.tensor_tensor(out=dm[:], in0=dm[:], in1=dm[:], op=OP.mult)
            var = ftile()
            nc.vector.tensor_tensor(out=var[:], in0=m2e, in1=m2o, op=OP.add)
            nc.vector.tensor_scalar_mul(var[:], var[:], 1.0 / D)
            nc.vector.tensor_scalar_mul(dm[:], dm[:], 0.25)
            nc.vector.tensor_tensor(out=var[:], in0=var[:], in1=dm[:], op=OP.add)
            return mean, var

        mx, vx = mean_var(xst)

        def mean_var_sums(sy, syy):
            # mean/var from ACT-accumulated sums: m = S/D, v = SS/D - m^2
            m = ftile()
            nc.vector.tensor_scalar_mul(m[:], sy[:], 1.0 / D)
            v = ftile()
            nc.vector.tensor_scalar_mul(v[:], syy[:], 1.0 / D)
            msq = ftile()
            nc.vector.tensor_tensor(out=msq[:], in0=m[:], in1=m[:], op=OP.mult)
            nc.vector.tensor_tensor(out=v[:], in0=v[:], in1=msq[:],
                                    op=OP.subtract)
            return m, v

        m1, v1 = mean_var_sums(sy1, syy1)
        m2, v2 = mean_var_sums(sy2, syy2)

        def pearson(sxy, my, vy):
            num = ftile()
            nc.vector.tensor_scalar_mul(num[:], sxy[:], 1.0 / D)
            t = ftile()
            nc.vector.tensor_tensor(out=t[:], in0=mx[:], in1=my[:], op=OP.mult)
            nc.vector.tensor_tensor(out=num[:], in0=num[:], in1=t[:],
                                    op=OP.subtract)
            den = ftile()
            nc.vector.tensor_tensor(out=den[:], in0=vx[:], in1=vy[:], op=OP.mult)
            nc.scalar.sqrt(den[:], den[:])
            nc.vector.reciprocal(out=den[:], in_=den[:])
            nc.vector.tensor_tensor(out=num[:], in0=num[:], in1=den[:],
                                    op=OP.mult)
            return num

        s1v = pearson(sxy1, m1, v1)
        s2v = pearson(sxy2, m2, v2)

        cmp = ftile()
        nc.vector.tensor_tensor(out=cmp[:], in0=s1v[:], in1=s2v[:], op=OP.is_gt)
        headsf = ftile()
        nc.vector.tensor_copy(out=headsf[:], in_=heads_sb[:])
        tailsf = ftile()
        nc.vector.tensor_copy(out=tailsf[:], in_=tails_sb[:])
        # arithmetic blend: start = tails + (heads - tails) * (s1 > s2)
        # exact: cmp is 0/1 and entity ids are < 2^24
        startv = ftile()
        nc.vector.tensor_tensor(out=startv[:], in0=headsf[:], in1=tailsf[:],
                                op=OP.subtract)
        nc.vector.tensor_tensor(out=startv[:], in0=startv[:], in1=cmp[:],
                                op=OP.mult)
        nc.vector.tensor_tensor(out=startv[:], in0=startv[:], in1=tailsf[:],
                                op=OP.add)

        lsc = ftile()
        nc.vector.reciprocal(out=lsc[:], in_=zsum[:])

        # stage all outputs into packed SBUF tiles, then 2 clean DMAs
        outf_sb = fin.tile([P, 5, cols], F32)
        nc.vector.tensor_copy(out=outf_sb[:, 0, :], in_=s1v[:])
        nc.vector.tensor_copy(out=outf_sb[:, 1, :], in_=s2v[:])
        nc.vector.tensor_copy(out=outf_sb[:, 2, :], in_=lsc[:])
        nc.vector.tensor_copy(out=outf_sb[:, 3, :], in_=startv[:])
        nc.vector.tensor_copy(out=outf_sb[:, 4, :], in_=chosen[:, :, 0])
        outi_sb = fin.tile([P, 2, cols], I32)
        nc.vector.tensor_copy(out=outi_sb[:, 0, :], in_=chosen[:, :, 1])
        nc.vector.tensor_copy(out=outi_sb[:, 1, :], in_=chosen[:, :, 2])
        nc.sync.dma_start(out=outf, in_=outf_sb[:])
        nc.sync.dma_start(out=outi, in_=outi_sb[:])

    nc.compile()
    return nc


_NC = None


def _get_nc():
    global _NC
    if _NC is None:
        _NC = build_kernel()
    return _NC


def kernel(context_qa, embedding_table, heads, tails, action_space, logits):
    from concourse.bass_utils import run_bass_kernel_spmd

    ctx = np.ascontiguousarray(np.asarray(context_qa, dtype=np.float32))
    table = np.ascontiguousarray(np.asarray(embedding_table, dtype=np.float32))
    heads_i = np.ascontiguousarray(np.asarray(heads).astype(np.int32))
    tails_i = np.ascontiguousarray(np.asarray(tails).astype(np.int32))
    lg = np.ascontiguousarray(np.asarray(logits, dtype=np.float32)[T - 1])
    asl = np.ascontiguousarray(
        np.asarray(action_space, dtype=np.int32)[T - 1].reshape(B, 4 * A))

    nc = _get_nc()
    in_maps = []
    for c in range(N_CORES):
        sl = slice(c * B_LOC, (c + 1) * B_LOC)
        in_maps.append({
            "x": ctx[sl], "table": table, "heads": heads_i[sl],
            "tails": tails_i[sl], "lg": lg[sl], "asl": asl[sl],
        })
    res = run_bass_kernel_spmd(nc, in_maps, list(range(N_CORES)))

    # device layout [P, n, cols] with row = p*cols + c -> [n, B_LOC]
    outf = np.concatenate(
        [res.results[c]["outf"].transpose(1, 0, 2).reshape(5, B_LOC)
         for c in range(N_CORES)], axis=1)
    outi = np.concatenate(
        [res.results[c]["outi"].transpose(1, 0, 2).reshape(2, B_LOC)
         for c in range(N_CORES)], axis=1)
    out = outf[:3].astype(np.float32)
    start_entities = outf[3].astype(np.float32)
    current_entities = outf[4].astype(np.float32)
    current_timestamps = outi[0].astype(np.int32)
    current_timestamps2 = outi[1].astype(np.int32)
    return (out, start_entities, current_entities, current_timestamps,
            current_timestamps2)


# revision 11
# speedup vs baseline: 1.2259x; 1.0058x over previous
"""Trainium2 Bass kernel for nn_Episode_91164975825422.

Computation (see module docstring in reference):
  - e1 = table[heads], e2 = table[tails]    (embedding gathers, [B,256])
  - s1 = pearson(context, e1), s2 = pearson(context, e2)
  - start_entities = where(s1 > s2, heads, tails) as f32
  - softmax/argmax over logits[-1] -> last_scores = 1/sum(exp(l-max))
  - chosen = action_space[-1][b, argmax_b, 1:4] -> current_entities/timestamps

Sharding: pure data parallel, batch B=65536 split over 8 cores (8192 rows
each); the [V,D] embedding table is replicated. Only the last timestep of
logits / action_space is used by the reference, so only that slice is sent
to the devices.

Row mapping on each core: row = p*64 + c  (p = SBUF partition, c = column).

Engine split (per core), each ~130us so the pipeline stays balanced:
  GpSimd  : 128 indirect row gathers (serial SWDGE descriptor-gen floor)
  DVE     : bn_stats for x mean+var, mul+reduce for sum(x*e1)/sum(x*e2),
            softmax max/denoms/argmax-mask, masked chosen-field sums
  ACT     : e1/e2 sums and sums-of-squares (Copy/Square with accumulate),
            exp, int->float conversion of action_space
  Sync DMA: all direct loads/stores (HWDGE)
(tensor_tensor_reduce would fuse mul+reduce but faults this runtime.)
"""

import sys

sys.path.insert(0, "/opt/trn_rl_repo")

import numpy as np
from contextlib import ExitStack

import concourse.bass as bass
import concourse.tile as tile
from concourse import bacc, mybir

N_CORES = 8
B = 65536
D = 256
V = 125726
A = 50
T = 4
P = 128

B_LOC = B // N_CORES          # 8192 rows per core
C = B_LOC // P                # 64 row-columns per partition
SUB = 8                       # rows-per-partition processed per group
GROUPS = C // SUB             # 8 groups

F32 = mybir.dt.float32
I32 = mybir.dt.int32
AX = mybir.AxisListType
OP = mybir.AluOpType
AF = mybir.ActivationFunctionType


def _bcast(ap, axis, count):
    """Insert a stride-0 (broadcast) axis into an AP at position `axis`."""
    return bass.AP(
        tensor=ap.tensor,
        offset=ap.offset,
        ap=list(ap.ap[:axis]) + [[0, count]] + list(ap.ap[axis:]),
    )


def build_kernel(b_loc=B_LOC, sub=SUB):
    cols = b_loc // P
    groups = cols // sub
    nc = bacc.Bacc("TRN2", target_bir_lowering=False, debug=False,
                   num_devices=N_CORES)

    x_d = nc.dram_tensor("x", [b_loc, D], F32, kind="ExternalInput").ap()
    tab_d = nc.dram_tensor("table", [V, D], F32, kind="ExternalInput").ap()
    hd_d = nc.dram_tensor("heads", [b_loc], I32, kind="ExternalInput").ap()
    tl_d = nc.dram_tensor("tails", [b_loc], I32, kind="ExternalInput").ap()
    lg_d = nc.dram_tensor("lg", [b_loc, A], F32, kind="ExternalInput").ap()
    as_d = nc.dram_tensor("asl", [b_loc, 4 * A], I32, kind="ExternalInput").ap()
    # outputs packed [P, n, cols] so the store is one contiguous-per-
    # partition DMA; the host unpacks (row = p*cols + c)
    outf = nc.dram_tensor("outf", [P, 5, b_loc // P], F32,
                          kind="ExternalOutput").ap()
    outi = nc.dram_tensor("outi", [P, 2, b_loc // P], I32,
                          kind="ExternalOutput").ap()

    x_r = x_d.rearrange("(p c) d -> p c d", p=P)
    lg_r = lg_d.rearrange("(p c) a -> p c a", p=P)
    as_r = as_d.rearrange("(p c) k -> p c k", p=P)
    hd_r = hd_d.rearrange("(p c) -> p c", p=P)
    tl_r = tl_d.rearrange("(p c) -> p c", p=P)

    with tile.TileContext(nc) as tc, ExitStack() as ctx:
        persist = ctx.enter_context(tc.tile_pool(name="persist", bufs=1))
        gin = ctx.enter_context(tc.tile_pool(name="gin", bufs=4))
        asp = ctx.enter_context(tc.tile_pool(name="asp", bufs=2))
        gtmp = ctx.enter_context(tc.tile_pool(name="gtmp", bufs=2))
        prodp = ctx.enter_context(tc.tile_pool(name="prod", bufs=6))
        fin = ctx.enter_context(tc.tile_pool(name="fin", bufs=1))

        heads_sb = persist.tile([P, cols], I32)
        tails_sb = persist.tile([P, cols], I32)
        nc.sync.dma_start(out=heads_sb[:], in_=hd_r)
        nc.sync.dma_start(out=tails_sb[:], in_=tl_r)

        xst = persist.tile([P, cols, 6], F32)     # bn_stats of context rows
        sy1 = persist.tile([P, cols], F32)        # ACT: sum(e1) per row
        syy1 = persist.tile([P, cols], F32)       # ACT: sum(e1^2) per row
        sy2 = persist.tile([P, cols], F32)        # ACT: sum(e2) per row
        syy2 = persist.tile([P, cols], F32)       # ACT: sum(e2^2) per row
        sxy1 = persist.tile([P, cols], F32)       # sum x*e1 per row
        sxy2 = persist.tile([P, cols], F32)
        zsum = persist.tile([P, cols], F32)       # softmax denominators
        chosen = persist.tile([P, cols, 3], F32)  # action fields 1..3 at argmax

        for g in range(groups):
            cs = slice(g * sub, (g + 1) * sub)

            xg = gin.tile([P, sub, D], F32, tag="xg")
            nc.sync.dma_start(out=xg[:], in_=x_r[:, cs, :])
            # HW only supports one offset per partition per indirect DMA
            e1g = gin.tile([P, sub, D], F32, tag="e1g")
            e2g = gin.tile([P, sub, D], F32, tag="e2g")
            for s in range(sub):
                c0 = g * sub + s
                nc.gpsimd.indirect_dma_start(
                    out=e1g[:, s, :], out_offset=None, in_=tab_d,
                    in_offset=bass.IndirectOffsetOnAxis(
                        ap=heads_sb[:, c0:c0 + 1], axis=0),
                )
                nc.gpsimd.indirect_dma_start(
                    out=e2g[:, s, :], out_offset=None, in_=tab_d,
                    in_offset=bass.IndirectOffsetOnAxis(
                        ap=tails_sb[:, c0:c0 + 1], axis=0),
                )
            lgt = gin.tile([P, sub, A], F32, tag="lgt")
            nc.sync.dma_start(out=lgt[:], in_=lg_r[:, cs, :])
            asgi = asp.tile([P, sub, 4 * A], I32, tag="asgi")
            nc.sync.dma_start(out=asgi[:], in_=as_r[:, cs, :])

            # --- Pearson stats ---
            # e1/e2 sums + squares on the Scalar engine, x stats on DVE
            for h in range(sub):
                c0 = g * sub + h
                nc.vector.bn_stats(out=xst[:, c0, :], in_=xg[:, h, :])
                sq1 = prodp.tile([P, D], F32, tag="sq")
                nc.scalar.activation(out=sq1[:], in_=e1g[:, h, :],
                                     func=AF.Square,
                                     accum_out=syy1[:, c0:c0 + 1])
                cp1 = prodp.tile([P, D], F32, tag="sq")
                nc.scalar.activation(out=cp1[:], in_=e1g[:, h, :],
                                     func=AF.Copy,
                                     accum_out=sy1[:, c0:c0 + 1])
                sq2 = prodp.tile([P, D], F32, tag="sq")
                nc.scalar.activation(out=sq2[:], in_=e2g[:, h, :],
                                     func=AF.Square,
                                     accum_out=syy2[:, c0:c0 + 1])
                cp2 = prodp.tile([P, D], F32, tag="sq")
                nc.scalar.activation(out=cp2[:], in_=e2g[:, h, :],
                                     func=AF.Copy,
                                     accum_out=sy2[:, c0:c0 + 1])
            # sum(x*e): elementwise product into scratch (keeps DVE
            # independent of the ACT stat passes) then 3D row-reduce
            xf = xg[:].rearrange("p s d -> p (s d)")
            pr1 = gtmp.tile([P, sub, D], F32, tag="pr")
            nc.vector.tensor_tensor(out=pr1[:].rearrange("p s d -> p (s d)"),
                                    in0=xf, in1=e1g[:].rearrange("p s d -> p (s d)"),
                                    op=OP.mult)
            nc.vector.tensor_reduce(out=sxy1[:, cs], in_=pr1[:], axis=AX.X,
                                    op=OP.add)
            pr2 = gtmp.tile([P, sub, D], F32, tag="pr")
            nc.vector.tensor_tensor(out=pr2[:].rearrange("p s d -> p (s d)"),
                                    in0=xf, in1=e2g[:].rearrange("p s d -> p (s d)"),
                                    op=OP.mult)
            nc.vector.tensor_reduce(out=sxy2[:, cs], in_=pr2[:], axis=AX.X,
                                    op=OP.add)

            # --- softmax max / denom / argmax mask ---
            mg = gtmp.tile([P, sub], F32, tag="mg")
            nc.vector.tensor_reduce(out=mg[:], in_=lgt[:], axis=AX.X, op=OP.max)
            mg_b = _bcast(mg[:], 2, A)
            lsub = gtmp.tile([P, sub, A], F32, tag="lsub")
            nc.vector.tensor_tensor(out=lsub[:], in0=lgt[:], in1=mg_b,
                                    op=OP.subtract)
            ex = gtmp.tile([P, sub, A], F32, tag="ex")
            nc.scalar.activation(out=ex[:], in_=lsub[:], func=AF.Exp)
            nc.vector.tensor_reduce(out=zsum[:, cs], in_=ex[:], axis=AX.X,
                                    op=OP.add)
            eq = gtmp.tile([P, sub, A], F32, tag="eq")
            nc.vector.tensor_scalar(out=eq[:], in0=lsub[:], scalar1=0.0,
                                    scalar2=None, op0=OP.is_equal)

            # --- chosen action fields via masked sum ---
            # asgi viewed as [P, sub, field k(4), action a(50)]; take k=1..3.
            # int32 in0 with f32 in1/out works natively on DVE.
            as_kv = asgi[:].rearrange("p s (a k) -> p s k a", k=4)[:, :, 1:4, :]
            eq_b = _bcast(eq[:], 2, 3)
            tt3 = gtmp.tile([P, sub, 3, A], F32, tag="tt3")
            nc.vector.tensor_tensor(out=tt3[:], in0=as_kv, in1=eq_b, op=OP.mult)
            nc.vector.tensor_reduce(out=chosen[:, cs, :], in_=tt3[:], axis=AX.X,
                                    op=OP.add)

        # ---------------- final per-row math on [P, cols] ----------------
        _fc = [0]

        def ftile():
            _fc[0] += 1
            return fin.tile([P, cols], F32, name=f"fin{_fc[0]}",
                            tag=f"fin{_fc[0]}")

        def mean_var(st):
            me, mo = st[:, :, 1], st[:, :, 4]
            m2e, m2o = st[:, :, 2], st[:, :, 5]
            mean = ftile()
            nc.vector.tensor_tensor(out=mean[:], in0=me, in1=mo, op=OP.add)
            nc.vector.tensor_scalar_mul(mean[:], mean[:], 0.5)
            dm = ftile()
            nc.vector.tensor_tensor(out=dm[:], in0=me, in1=mo, op=OP.subtract)
            nc.vector.tensor_tensor(out=dm[:], in0=dm[:], in1=dm[:], op=OP.mult)
            var = ftile()
            nc.vector.tensor_tensor(out=var[:], in0=m2e, in1=m2o, op=OP.add)
            nc.vector.tensor_scalar_mul(var[:], var[:], 1.0 / D)
            nc.vector.tensor_scalar_mul(dm[:], dm[:], 0.25)
            nc.vector.tensor_tensor(out=var[:], in0=var[:], in1=dm[:], op=OP.add)
            return mean, var

        mx, vx = mean_var(xst)

        def mean_var_sums(sy, syy):
            # mean/var from ACT-accumulated sums: m = S/D, v = SS/D - m^2
            m = ftile()
            nc.vector.tensor_scalar_mul(m[:], sy[:], 1.0 / D)
            v = ftile()
            nc.vector.tensor_scalar_mul(v[:], syy[:], 1.0 / D)
            msq = ftile()
            nc.vector.tensor_tensor(out=msq[:], in0=m[:], in1=m[:], op=OP.mult)
            nc.vector.tensor_tensor(out=v[:], in0=v[:], in1=msq[:],
                                    op=OP.subtract)
            return m, v

        m1, v1 = mean_var_sums(sy1, syy1)
        m2, v2 = mean_var_sums(sy2, syy2)

        def pearson(sxy, my, vy):
            num = ftile()
            nc.vector.tensor_scalar_mul(num[:], sxy[:], 1.0 / D)
            t = ftile()
            nc.vector.tensor_tensor(out=t[:], in0=mx[:], in1=my[:], op=OP.mult)
            nc.vector.tensor_tensor(out=num[:], in0=num[:], in1=t[:],
                                    op=OP.subtract)
            den = ftile()
            nc.vector.tensor_tensor(out=den[:], in0=vx[:], in1=vy[:], op=OP.mult)
            nc.scalar.sqrt(den[:], den[:])
            nc.vector.reciprocal(out=den[:], in_=den[:])
            nc.vector.tensor_tensor(out=num[:], in0=num[:], in1=den[:],
                                    op=OP.mult)
            return num

        s1v = pearson(sxy1, m1, v1)
        s2v = pearson(sxy2, m2, v2)

        cmp = ftile()
        nc.vector.tensor_tensor(out=cmp[:], in0=s1v[:], in1=s2v[:], op=OP.is_gt)
        headsf = ftile()
        nc.vector.tensor_copy(out=headsf[:], in_=heads_sb[:])
        tailsf = ftile()
        nc.vector.tensor_copy(out=tailsf[:], in_=tails_sb[:])
        # arithmetic blend: start = tails + (heads - tails) * (s1 > s2)
        # exact: cmp is 0/1 and entity ids are < 2^24
        startv = ftile()
        nc.vector.tensor_tensor(out=startv[:], in0=headsf[:], in1=tailsf[:],
                                op=OP.subtract)
        nc.vector.tensor_tensor(out=startv[:], in0=startv[:], in1=cmp[:],
                                op=OP.mult)
        nc.vector.tensor_tensor(out=startv[:], in0=startv[:], in1=tailsf[:],
                                op=OP.add)

        lsc = ftile()
        nc.vector.reciprocal(out=lsc[:], in_=zsum[:])

        # stage all outputs into packed SBUF tiles, then 2 clean DMAs
        outf_sb = fin.tile([P, 5, cols], F32)
        nc.vector.tensor_copy(out=outf_sb[:, 0, :], in_=s1v[:])
        nc.vector.tensor_copy(out=outf_sb[:, 1, :], in_=s2v[:])
        nc.vector.tensor_copy(out=outf_sb[:, 2, :], in_=lsc[:])
        nc.vector.tensor_copy(out=outf_sb[:, 3, :], in_=startv[:])
        nc.vector.tensor_copy(out=outf_sb[:, 4, :], in_=chosen[:, :, 0])
        outi_sb = fin.tile([P, 2, cols], I32)
        nc.vector.tensor_copy(out=outi_sb[:, 0, :], in_=chosen[:, :, 1])
        nc.vector.tensor_copy(out=outi_sb[:, 1, :], in_=chosen[:, :, 2])
        nc.sync.dma_start(out=outf, in_=outf_sb[:])
        nc.sync.dma_start(out=outi, in_=outi_sb[:])

    nc.compile()
    return nc


_NC = None


def _get_nc():
    global _NC
    if _NC is None:
        _NC = build_kernel()
    return _NC


def kernel(context_qa, embedding_table, heads, tails, action_space, logits):
    from concourse.bass_utils import run_bass_kernel_spmd

    ctx = np.ascontiguousarray(np.asarray(context_qa, dtype=np.float32))
    table = np.ascontiguousarray(np.asarray(embedding_table, dtype=np.float32))
    heads_i = np.ascontiguousarray(np.asarray(heads).astype(np.int32))
    tails_i = np.ascontiguousarray(np.asarray(tails).astype(np.int32))
    lg = np.ascontiguousarray(np.asarray(logits, dtype=np.float32)[T - 1])
    asl = np.ascontiguousarray(
        np.asarray(action_space, dtype=np.int32)[T - 1].reshape(B, 4 * A))

    nc = _get_nc()
    in_maps = []
    for c in range(N_CORES):
        sl = slice(c * B_LOC, (c + 1) * B_LOC)
        in_maps.append({
            "x": ctx[sl], "table": table, "heads": heads_i[sl],
            "tails": tails_i[sl], "lg": lg[sl], "asl": asl[sl],
        })
    res = run_bass_kernel_spmd(nc, in_maps, list(range(N_CORES)))

    # device layout [P, n, cols] with row = p*cols + c -> [n, B_LOC]
    outf = np.concatenate(
        [res.results[c]["outf"].transpose(1, 0, 2).reshape(5, B_LOC)
         for c in range(N_CORES)], axis=1)
    outi = np.concatenate(
        [res.results[c]["outi"].transpose(1, 0, 2).reshape(2, B_LOC)
         for c in range(N_CORES)], axis=1)
    out = outf[:3].astype(np.float32)
    start_entities = outf[3].astype(np.float32)
    current_entities = outf[4].astype(np.float32)
    current_timestamps = outi[0].astype(np.int32)
    current_timestamps2 = outi[1].astype(np.int32)
    return (out, start_entities, current_entities, current_timestamps,
            current_timestamps2)
